# revision 1
# baseline (speedup 1.0000x reference)
"""LucidLinearAttention Trainium2 kernel (8-core SPMD).

Sharding: batch b = core//2 (4 batches), head-group hg = core%2 (8 heads each).
Each core computes qkv projection for its heads, chunked linear attention
(bucket-exclusive cumsum) via a hybrid block-causal formulation, and its
partial output projection. Host sums the two head-group partials per batch.

All matmul accumulation groups use lhsT/rhs at partition base 0 with uniform
K (mixed-base accumulation groups crash the HW - validated by bisection).
"""
import sys
import numpy as np

for p in ("/opt/trn_rl_repo", "/root/.axon_site/_ro/trn_rl_repo"):
    if p not in sys.path:
        sys.path.insert(0, p)

import concourse.mybir as mybir
import concourse.tile as tile
from concourse import bacc
from concourse.bass_utils import run_bass_kernel_spmd
from concourse.masks import make_identity

F32 = mybir.dt.float32
F32R = mybir.dt.float32r
EXP = mybir.ActivationFunctionType.Exp

B, T, D = 4, 4096, 1024
NH, HD, BUCKET = 16, 64, 64
HPC = 8            # heads per core
GD = HPC * HD      # 512 group dim
NBLK = 8           # coarse blocks
BT = T // NBLK     # 512 rows per block
NC_CORES = 8

_CACHE = {}


def _build():
    nc = bacc.Bacc("TRN2", target_bir_lowering=False, debug=False,
                   num_devices=NC_CORES)
    xT = nc.dram_tensor("xT", [D, T], F32, kind="ExternalInput").ap()
    wqT = nc.dram_tensor("wqT", [D, GD], F32, kind="ExternalInput").ap()
    wkT = nc.dram_tensor("wkT", [D, GD], F32, kind="ExternalInput").ap()
    wvT = nc.dram_tensor("wvT", [D, GD], F32, kind="ExternalInput").ap()
    woT = nc.dram_tensor("woT", [GD, D], F32, kind="ExternalInput").ap()
    y = nc.dram_tensor("y", [T, D], F32, kind="ExternalOutput").ap()

    with tile.TileContext(nc) as tc:
        with nc.allow_low_precision(reason="float32r matmul rounding by design"), \
             tc.tile_pool(name="w", bufs=1) as wp, \
             tc.tile_pool(name="per", bufs=1) as pp, \
             tc.tile_pool(name="sb", bufs=1) as sbp, \
             tc.tile_pool(name="ps", bufs=1, space="PSUM") as ps:

            # ---- resident weights -------------------------------------
            wq_sb = [wp.tile([128, GD], F32R, tag=f"wq{dc}", name=f"wq{dc}") for dc in range(8)]
            wk_sb = [wp.tile([128, GD], F32R, tag=f"wk{dc}", name=f"wk{dc}") for dc in range(8)]
            wv_sb = [wp.tile([128, GD], F32R, tag=f"wv{dc}", name=f"wv{dc}") for dc in range(8)]
            wo_sb = [wp.tile([64, D], F32R, tag=f"wo{h}", name=f"wo{h}") for h in range(HPC)]
            for dc in range(8):
                for src_ap, dst in ((wqT, wq_sb), (wkT, wk_sb), (wvT, wv_sb)):
                    stg = sbp.tile([128, GD], F32, tag="stage", name="stage", bufs=2)
                    nc.sync.dma_start(stg[:], src_ap[128 * dc:128 * (dc + 1), :])
                    nc.vector.tensor_copy(dst[dc][:], stg[:])
            for h in range(HPC):
                stg = sbp.tile([64, D], F32, tag="wstage", name="wstage", bufs=2)
                nc.sync.dma_start(stg[:], woT[64 * h:64 * (h + 1), :])
                nc.vector.tensor_copy(wo_sb[h][:], stg[:])

            # ---- persistent state -------------------------------------
            ident = pp.tile([128, 128], F32, tag="ident")
            make_identity(nc, ident[:])
            ident_r = pp.tile([128, 128], F32R, tag="ident_r")
            nc.vector.tensor_copy(ident_r[:], ident[:])
            # F32 staging constants (memset on F32R is invalid ISA; fp32r
            # tiles must be produced by rounding compute instructions).
            zero_f32 = pp.tile([128, BT], F32, tag="zero_f32")
            nc.vector.memset(zero_f32[:], 0.0)
            one_f32 = pp.tile([128, 16], F32, tag="one_f32")
            nc.vector.memset(one_f32[:], 1.0)
            # bvec: K=2 broadcast weights; row 64 = 1, row 65 = 0.
            bv_f32 = pp.tile([66, 64], F32, tag="bv_f32")
            nc.vector.memset(bv_f32[:], 0.0)
            nc.vector.memset(bv_f32[64:65, :], 1.0)
            bvec = pp.tile([66, 64], F32R, tag="bvec")
            nc.vector.tensor_copy(bvec[:], bv_f32[:])
            # qtu_h: [128, BT]; rows 0-63 = exp(q) of head h (d x t),
            # rows 64-127 permanently zero (K=128 inter matmul padding).
            qtu = [pp.tile([128, BT], F32R, tag=f"qtu{h}", name=f"qtu{h}") for h in range(HPC)]
            for h in range(HPC):
                nc.vector.tensor_copy(qtu[h][:], zero_f32[:])
            # caug_h: [128, 66]; rows 0-63 = [C (d x e) | kcum | pad], rest 0.
            caug = [pp.tile([128, 66], F32R, tag=f"caug{h}", name=f"caug{h}") for h in range(HPC)]
            for h in range(HPC):
                nc.vector.tensor_copy(caug[h][:], zero_f32[:, 0:66])
            # vaug[tc]: [128, 8*66]; per head h cols h*66..h*66+64 = V,
            # col h*66+64 = ones (den trick), col h*66+65 = zero pad.
            vaug = [pp.tile([128, HPC * 66], F32R, tag=f"vaug{t}", name=f"vaug{t}") for t in range(4)]
            one_col = one_f32[:].rearrange("p (a b) -> p a b", b=1)[:, 0:8, :]
            zero_col = zero_f32[:, 0:8].rearrange("p (a b) -> p a b", b=1)
            for t4 in range(4):
                vv = vaug[t4][:].rearrange("p (h c) -> p h c", c=66)
                nc.vector.tensor_copy(vv[:, :, 64:65], one_col)
                nc.vector.tensor_copy(vv[:, :, 65:66], zero_col)
            # ssb: 2 parity sets x 4 chunks of masked S^T [128, BT].
            # Zero strips are preset once and never overwritten.
            ssb = [[pp.tile([128, BT], F32R, tag=f"ssb{s}_{t}", name=f"ssb{s}_{t}") for t in range(4)]
                   for s in range(2)]
            for s in range(2):
                for t4 in range(4):
                    nc.vector.tensor_copy(ssb[s][t4][:], zero_f32[:])

            # ---- main loop over coarse blocks -------------------------
            for ct in range(NBLK):
                t0 = ct * BT
                # x^T tiles for this block: [d-chunk 128, t 512]
                xsb = [sbp.tile([128, BT], F32R, tag=f"xsb{dc}", name=f"xsb{dc}") for dc in range(8)]
                for dc in range(8):
                    xstg = sbp.tile([128, BT], F32, tag="xstage", name="xstage", bufs=2)
                    nc.sync.dma_start(
                        xstg[:], xT[128 * dc:128 * (dc + 1), t0:t0 + BT])
                    nc.vector.tensor_copy(xsb[dc][:], xstg[:])

                # Q^T projection per head (M=64) + exp
                for h in range(HPC):
                    pq = ps.tile([64, BT], F32, tag="big")
                    for dc in range(8):
                        nc.tensor.matmul(
                            pq[:], wq_sb[dc][:, 64 * h:64 * (h + 1)], xsb[dc][:],
                            start=(dc == 0), stop=(dc == 7))
                    nc.scalar.activation(qtu[h][0:64, :], pq[:], EXP)

                # K natural projection per t-chunk (M=128) + exp
                ksb = [sbp.tile([128, GD], F32R, tag=f"ksb{t}", name=f"ksb{t}") for t in range(4)]
                for t4 in range(4):
                    pk = ps.tile([128, GD], F32, tag="big")
                    for dc in range(8):
                        nc.tensor.matmul(
                            pk[:], xsb[dc][:, 128 * t4:128 * (t4 + 1)], wk_sb[dc][:],
                            start=(dc == 0), stop=(dc == 7))
                    nc.scalar.activation(ksb[t4][:], pk[:], EXP)

                # V projection per t-chunk -> vaug strided cols
                for t4 in range(4):
                    pv = ps.tile([128, GD], F32, tag="big")
                    for dc in range(8):
                        nc.tensor.matmul(
                            pv[:], xsb[dc][:, 128 * t4:128 * (t4 + 1)], wv_sb[dc][:],
                            start=(dc == 0), stop=(dc == 7))
                    vv = vaug[t4][:].rearrange("p (h c) -> p h c", c=66)
                    pvv = pv[:].rearrange("p (h c) -> p h c", c=64)
                    nc.vector.tensor_copy(vv[:, :, 0:64], pvv[:, :, :])

                # K^T per head via PE transpose: kt_h [64, BT]
                kt = [sbp.tile([64, BT], F32R, tag=f"kt{h}", name=f"kt{h}") for h in range(HPC)]
                for h in range(HPC):
                    for t4 in range(4):
                        pt = ps.tile([64, 128], F32R, tag="small")
                        nc.tensor.transpose(
                            pt[:], ksb[t4][:, 64 * h:64 * (h + 1)], ident_r[:])
                        nc.vector.tensor_copy(
                            kt[h][:, 128 * t4:128 * (t4 + 1)], pt[:])

                # ---- attention per head -------------------------------
                xots = []
                for h in range(HPC):
                    par = h % 2
                    # S^T chunks + masked region copies
                    for t4 in range(4):
                        pst = ps.tile([128, BT], F32, tag="s")
                        nc.tensor.matmul(
                            pst[:], kt[h][:, 128 * t4:128 * (t4 + 1)],
                            qtu[h][0:64, :], start=True, stop=True)
                        c0 = (2 * t4 + 1) * 64
                        c1 = (2 * t4 + 2) * 64
                        nc.scalar.copy(ssb[par][t4][0:64, c0:BT], pst[0:64, c0:BT])
                        if c1 < BT:
                            nc.scalar.copy(
                                ssb[par][t4][64:128, c1:BT], pst[64:128, c1:BT])

                    # OUT group: inter (K=128, zero-padded) + 4 intra partial-N
                    po = ps.tile([66, BT], F32, tag="o")
                    nc.tensor.matmul(po[:], caug[h][:, :], qtu[h][:, :],
                                     start=True, stop=False)
                    for t4 in range(4):
                        n0 = (2 * t4 + 1) * 64
                        nc.tensor.matmul(
                            po[0:66, n0:BT],
                            vaug[t4][:, 66 * h:66 * h + 66],
                            ssb[par][t4][:, n0:BT],
                            start=False, stop=(t4 == 3))

                    # normalize: dinv = 1/max(den,eps); bcast via K=1 matmul
                    dv = sbp.tile([66, BT], F32R, tag="dv")
                    nc.vector.tensor_scalar_max(dv[64:66, :], po[64:66, :], 1e-30)
                    nc.vector.reciprocal(dv[64:66, :], dv[64:66, :])
                    pb = ps.tile([64, BT], F32, tag="small")
                    nc.tensor.matmul(pb[:], bvec[64:66, 0:64], dv[64:66, :],
                                     start=True, stop=True)
                    sbb = sbp.tile([64, BT], F32, tag="sbb")
                    nc.scalar.copy(sbb[:], pb[:])
                    xot = sbp.tile([64, BT], F32R, tag=f"xot{h}")
                    nc.vector.tensor_mul(xot[:], po[0:64, :], sbb[:])

                    # C/kcum update (after inter read): caug += K^T @ V_aug
                    pc = ps.tile([64, 66], F32, tag="small")
                    for t4 in range(4):
                        nc.tensor.matmul(
                            pc[:], ksb[t4][:, 64 * h:64 * (h + 1)],
                            vaug[t4][:, 66 * h:66 * h + 66],
                            start=(t4 == 0), stop=(t4 == 3))
                    nc.vector.tensor_add(caug[h][0:64, :], caug[h][0:64, :], pc[:])

                    xots.append(xot)

                # partial output projection + store y block
                for t4 in range(4):
                    for fc in range(2):
                        py = ps.tile([128, GD], F32, tag="big")
                        for h in range(HPC):
                            nc.tensor.matmul(
                                py[:],
                                xots[h][:, 128 * t4:128 * (t4 + 1)],
                                wo_sb[h][:, GD * fc:GD * (fc + 1)],
                                start=(h == 0), stop=(h == HPC - 1))
                        ysb = sbp.tile([128, GD], F32, tag="ysb")
                        nc.vector.tensor_copy(ysb[:], py[:])
                        nc.sync.dma_start(
                            y[t0 + 128 * t4:t0 + 128 * (t4 + 1),
                              GD * fc:GD * (fc + 1)], ysb[:])

    nc.compile()
    return nc


def _get_nc():
    if "nc" not in _CACHE:
        _CACHE["nc"] = _build()
    return _CACHE["nc"]


def kernel(x, W_qkv, W_out):
    x = np.asarray(x, dtype=np.float32)
    W_qkv = np.asarray(W_qkv, dtype=np.float32)
    W_out = np.asarray(W_out, dtype=np.float32)
    nc = _get_nc()

    xTs = [np.ascontiguousarray(x[b].T) for b in range(B)]
    in_maps = []
    for c in range(NC_CORES):
        b, hg = c // 2, c % 2
        s = slice(hg * GD, (hg + 1) * GD)
        in_maps.append({
            "xT": xTs[b],
            "wqT": np.ascontiguousarray(W_qkv[0 * D:1 * D][s].T),
            "wkT": np.ascontiguousarray(W_qkv[1 * D:2 * D][s].T),
            "wvT": np.ascontiguousarray(W_qkv[2 * D:3 * D][s].T),
            "woT": np.ascontiguousarray(W_out[:, s].T),
        })
    res = run_bass_kernel_spmd(nc, in_maps, core_ids=list(range(NC_CORES)))
    out = np.empty((B, T, D), dtype=np.float32)
    for b in range(B):
        out[b] = res.results[2 * b]["y"] + res.results[2 * b + 1]["y"]
    return out

